# revision 1
# baseline (speedup 1.0000x reference)
"""Trainium2 Bass kernel for nn_AttModel (B=8, S=96, D=768, R=24, RSEQ=8, TAG=3).

Data-parallel over batch: core i handles sample i.
Per-core program (one sample):
  1. refine scan in score space: s_{t+1} = s_t + (scale*A@A.T) @ softmax(s_t),
     b_final.T = b0.T + A.T @ sum_t softmax(s_t)   (A fixed across steps)
  2. H projections, feature-major: HhT/HtT [2304 x 96], proj_b folded into HhT
  3. pairwise loop: V = relu(HtT + HhT[:, i]) per k-tile (bf16), split across
     DVE (fused tensor_scalar add+max) and ACT (activation Relu with bias);
     out[72, i-block] = sum_k relW[k].T @ V[k] accumulated in PSUM.
Output per core: [72, 9216] with channel c = tag*24 + rel (rel_W pre-permuted
on host), reshaped on host to [3, 24, 96, 96].
"""
import sys

sys.path.insert(0, "/opt/trn_rl_repo")

import numpy as np

S, D, H3 = 96, 768, 2304
R, RSEQ, TAG, C = 24, 8, 3, 72
B = 8
KT = D // 128          # 6 k-tiles over D
MT = H3 // 128         # 18 m-tiles over 3D
IGRP = 4               # i's per output group
NG = S // IGRP         # 24 groups
NFREE = IGRP * S       # 384 moving free dim
DVE_K_N = 14           # k-tiles produced on DVE; rest on ACT
SCALE = 1.0 / float(np.sqrt(np.float32(D)))


def build_nc(repeat: int = 1, skip_refine=False, skip_h=False, skip_main=False):
    import concourse.bass as bass
    from concourse import bacc, mybir
    import concourse.tile as tile
    from concourse.masks import make_identity

    f32 = mybir.dt.float32
    bf16 = mybir.dt.bfloat16
    AF = mybir.ActivationFunctionType
    ALU = mybir.AluOpType
    AX = mybir.AxisListType

    nc = bacc.Bacc()
    enc = nc.dram_tensor("enc", [S, D], f32, kind="ExternalInput")
    arel = nc.dram_tensor("arel", [RSEQ, D], f32, kind="ExternalInput")
    projW = nc.dram_tensor("projW", [2 * D, H3], f32, kind="ExternalInput")
    projb = nc.dram_tensor("projb", [H3], f32, kind="ExternalInput")
    relw = nc.dram_tensor("relw", [H3, C], f32, kind="ExternalInput")
    out = nc.dram_tensor("out", [C, S * S], f32, kind="ExternalOutput")

    dve_ks = list(range(DVE_K_N))
    act_ks = list(range(DVE_K_N, MT))

    with tile.TileContext(nc) as tc:
        with (
            tc.tile_pool(name="persist", bufs=1) as pp,
            tc.tile_pool(name="work", bufs=3) as wp,
            tc.tile_pool(name="vd", bufs=14) as vdp,
            tc.tile_pool(name="va", bufs=8) as vap,
            tc.tile_pool(name="pst", bufs=3, space="PSUM") as pst,
            tc.tile_pool(name="pso", bufs=4, space="PSUM") as pso,
            tc.tile_pool(name="pss", bufs=1, space="PSUM") as pss,
        ):

            def body(_it=None):
                # ---------- loads ----------
                ident = pp.tile([128, 128], f32, tag="ident")
                make_identity(nc, ident[:])

                enc_nat = wp.tile([S, D], f32, tag="enc_nat")
                nc.sync.dma_start(enc_nat[:], enc[:])
                a_nat = pp.tile([RSEQ, D], f32, tag="a_nat")
                nc.sync.dma_start(a_nat[:], arel[:])
                pb_sb = pp.tile([128, MT], f32, tag="pb")
                nc.sync.dma_start(
                    pb_sb[:], projb.rearrange("(t p) -> p t", p=128)
                )
                # proj_W resident: 12 tiles [128, H3]; one HWDGE queue fans out
                pw = []
                for kt in range(2 * KT):
                    t = pp.tile([128, H3], f32, tag=f"pw{kt}")
                    nc.sync.dma_start(t[:], projW[kt * 128:(kt + 1) * 128, :])
                    pw.append(t)
                # rel_W -> f32 staging -> bf16 persistent (needed only at main)
                rwr = []
                for k in range(MT):
                    stg = wp.tile([128, C], f32, tag="rw_stage")
                    nc.sync.dma_start(stg[:], relw[k * 128:(k + 1) * 128, :])
                    t = pp.tile([128, C], bf16, tag=f"rwr{k}")
                    nc.vector.tensor_scalar_mul(t[:], stg[:], 1.0)
                    rwr.append(t)

                # ---------- transposes ----------
                # b.T tiles [128, 96] (feature-major enc)
                bT = []
                for k in range(KT):
                    ps = pst.tile([128, S], f32, tag="ps_t")
                    nc.tensor.transpose(
                        ps[:], enc_nat[:, k * 128:(k + 1) * 128], ident[:S, :S]
                    )
                    t = pp.tile([128, S], f32, tag=f"bT{k}")
                    nc.scalar.copy(t[:], ps[:])
                    bT.append(t)
                # A.T tiles [128, 8], raw + pre-scaled
                at_raw, at_scl = [], []
                for k in range(KT):
                    ps = pst.tile([128, RSEQ], f32, tag="ps_t")
                    nc.tensor.transpose(
                        ps[:], a_nat[:, k * 128:(k + 1) * 128],
                        ident[:RSEQ, :RSEQ],
                    )
                    tr = pp.tile([128, RSEQ], f32, tag=f"atr{k}")
                    nc.scalar.copy(tr[:], ps[:])
                    ts = pp.tile([128, RSEQ], f32, tag=f"ats{k}")
                    nc.scalar.mul(ts[:], ps[:], SCALE)
                    at_raw.append(tr)
                    at_scl.append(ts)

                # ---------- refine scan (score space) ----------
                # G' = scale * A @ A.T  [8, 8]
                gps = pst.tile([RSEQ, RSEQ], f32, tag="ps_t")
                for k in range(KT):
                    nc.tensor.matmul(
                        gps[:], at_scl[k][:], at_raw[k][:],
                        start=(k == 0), stop=(k == KT - 1),
                    )
                g_sb = pp.tile([RSEQ, RSEQ], f32, tag="g")
                nc.vector.tensor_scalar_mul(g_sb[:], gps[:], 1.0)

                # s_psum = scale * A @ b0.T  [8, 96]; stays in PSUM all scan
                s_ps = pss.tile([RSEQ, S], f32, tag="s")
                for k in range(KT):
                    nc.tensor.matmul(
                        s_ps[:], at_scl[k][:], bT[k][:],
                        start=(k == 0), stop=False, skip_group_check=True,
                    )
                wsum = pp.tile([RSEQ, S], f32, tag="wsum")
                nc.vector.memset(wsum[:], 0.0)
                for t in range(0 if skip_refine else R):
                    negmax = wp.tile([RSEQ, 1], f32, tag="negmax")
                    nc.vector.reduce_max(
                        negmax[:], s_ps[:], axis=AX.X, negate=True
                    )
                    u = wp.tile([RSEQ, S], f32, tag="u")
                    rs = wp.tile([RSEQ, 1], f32, tag="rs")
                    nc.scalar.activation(
                        u[:], s_ps[:], AF.Exp, bias=negmax[:], scale=1.0,
                        accum_out=rs[:],
                    )
                    rinv = wp.tile([RSEQ, 1], f32, tag="rinv")
                    nc.vector.reciprocal(rinv[:], rs[:])
                    w = wp.tile([RSEQ, S], f32, tag="w")
                    nc.vector.tensor_scalar_mul(w[:], u[:], rinv[:])
                    nc.vector.tensor_tensor(
                        wsum[:], wsum[:], w[:], op=ALU.add
                    )
                    if t < R - 1:
                        nc.tensor.matmul(
                            s_ps[:], g_sb[:], w[:],
                            start=False, stop=(t == R - 2),
                            skip_group_check=True,
                        )
                # b_final.T = b0.T + A.T @ wsum
                for k in range(KT):
                    ps = pst.tile([128, S], f32, tag="ps_t")
                    nc.tensor.matmul(
                        ps[:], a_nat[:, k * 128:(k + 1) * 128], wsum[:],
                        start=True, stop=True,
                    )
                    nc.vector.tensor_tensor(
                        bT[k][:], bT[k][:], ps[:], op=ALU.add
                    )

                # ---------- H projections (feature-major, hh/ht per m) ----------
                hh, ht = [None] * MT, [None] * MT
                if skip_h:
                    for m in range(MT):
                        th = pp.tile([128, S], f32, tag=f"hh{m}")
                        nc.vector.memset(th[:], 0.01)
                        hh[m] = th
                        tt = pp.tile([128, S], bf16, tag=f"ht{m}")
                        nc.vector.memset(tt[:], 0.01)
                        ht[m] = tt
                for m in (range(MT) if not skip_h else []):
                    msl = slice(m * 128, (m + 1) * 128)
                    ps = pst.tile([128, S], f32, tag="ps_t")
                    for k in range(KT):
                        nc.tensor.matmul(
                            ps[:], pw[k][:, msl], bT[k][:],
                            start=(k == 0), stop=(k == KT - 1),
                        )
                    th = pp.tile([128, S], f32, tag=f"hh{m}")
                    # fold proj_b into HhT
                    nc.scalar.activation(
                        th[:], ps[:], AF.Identity,
                        bias=pb_sb[:, m:m + 1], scale=1.0,
                    )
                    hh[m] = th
                    ps2 = pst.tile([128, S], f32, tag="ps_t")
                    for k in range(KT):
                        nc.tensor.matmul(
                            ps2[:], pw[KT + k][:, msl], bT[k][:],
                            start=(k == 0), stop=(k == KT - 1),
                        )
                    tt = pp.tile([128, S], bf16, tag=f"ht{m}")
                    nc.vector.tensor_scalar_mul(tt[:], ps2[:], 1.0)
                    ht[m] = tt

                # ---------- pairwise main loop ----------
                for ig in range(0 if skip_main else NG):
                    ops = pso.tile([C, NFREE], f32, tag="ops")
                    vtiles = {}
                    for k in act_ks:
                        v = vap.tile([128, NFREE], bf16, tag="va")
                        for ii in range(IGRP):
                            i = ig * IGRP + ii
                            nc.scalar.activation(
                                v[:, ii * S:(ii + 1) * S], ht[k][:],
                                AF.Relu, bias=hh[k][:, i:i + 1], scale=1.0,
                            )
                        vtiles[k] = v
                    for k in dve_ks:
                        v = vdp.tile([128, NFREE], bf16, tag="vd")
                        for ii in range(IGRP):
                            i = ig * IGRP + ii
                            nc.vector.tensor_scalar(
                                v[:, ii * S:(ii + 1) * S], ht[k][:],
                                hh[k][:, i:i + 1], 0.0,
                                op0=ALU.add, op1=ALU.max,
                            )
                        vtiles[k] = v
                    order = dve_ks + act_ks
                    for j, k in enumerate(order):
                        nc.tensor.matmul(
                            ops[:], rwr[k][:], vtiles[k][:],
                            start=(j == 0), stop=(j == MT - 1),
                        )
                    ostg = wp.tile([C, NFREE], f32, tag="ostg")
                    if ig % 2 == 0:
                        nc.scalar.copy(ostg[:], ops[:])
                    else:
                        nc.vector.tensor_scalar_mul(ostg[:], ops[:], 1.0)
                    nc.sync.dma_start(
                        out[:, ig * NFREE:(ig + 1) * NFREE], ostg[:]
                    )

            if repeat == 1:
                body()
            else:
                with tc.For_i(0, repeat, 1) as it:
                    body(it)

    nc.finalize()
    return nc


_CACHED_NC = None


def _prep_in_maps(encoded_text, rel_types_encoded, proj_W, proj_b, rel_W):
    # permute rel_W columns: kernel channel c = tag*24 + rel reads original
    # column rel*3 + tag
    relw_perm = np.ascontiguousarray(
        rel_W.reshape(H3, R, TAG).transpose(0, 2, 1).reshape(H3, C)
    ).astype(np.float32)
    in_maps = []
    for i in range(B):
        in_maps.append({
            "enc": np.ascontiguousarray(encoded_text[i], dtype=np.float32),
            "arel": np.ascontiguousarray(
                rel_types_encoded[i], dtype=np.float32
            ),
            "projW": np.ascontiguousarray(proj_W, dtype=np.float32),
            "projb": np.ascontiguousarray(proj_b, dtype=np.float32),
            "relw": relw_perm,
        })
    return in_maps


def _assemble(results, rel_b):
    outs = []
    for i in range(B):
        o = results[i]["out"].reshape(TAG, R, S, S)
        outs.append(o)
    full = np.stack(outs, axis=0).astype(np.float32)  # [B, 3, 24, 96, 96]
    if np.any(rel_b):
        relb_perm = np.asarray(rel_b, dtype=np.float32).reshape(R, TAG).T
        full = full + relb_perm[None, :, :, None, None]
    return full


def kernel(encoded_text, rel_types_encoded, proj_W, proj_b, rel_W, rel_b):
    global _CACHED_NC
    from concourse.bass_utils import run_bass_kernel_spmd

    if _CACHED_NC is None:
        _CACHED_NC = build_nc(repeat=1)
    in_maps = _prep_in_maps(
        encoded_text, rel_types_encoded, proj_W, proj_b, rel_W
    )
    res = run_bass_kernel_spmd(_CACHED_NC, in_maps, list(range(B)))
    return _assemble(res.results, rel_b)



# revision 10
# speedup vs baseline: 1.2581x; 1.2581x over previous
"""Trainium2 Bass kernel for nn_AttModel (B=8, S=96, D=768, R=24, RSEQ=8, TAG=3).

Data-parallel over batch: core i handles sample i. v2 design:

  1. Host pre-converts proj_W / rel_W(permuted) / pair-selectors to bf16;
     W DMA is ~21us instead of 42us.
  2. Refine scan in score space with LAGGED max (reduce_max runs off the
     critical chain; exp(s_t - max(s_{t-1})) <= e^30, safe in f32) and the
     softmax normalizer folded into the tiny [8,8] G-scale instead of a
     [8,96] w-scale.
  3. H projections in bf16, two layouts:
     - feature-major hh/ht [128, 96] for k-tiles 0..5 (DVE-direct V build)
     - natural-layout combined tiles [128, 1536] for k-tiles 6..17:
       partitions 0..95 = Ht_nat(j), 96..127 = Hh_nat rows 32b..32b+31
  4. Main loop per group g (4 i's x 96 j's = 384 pairs):
     - k 6..17: pair-matmul  P = combined[:, kslice].T @ selR[g%8]  on PE,
       then relu PSUM->SBUF bf16 copy (k 6..15 on ACT, 16..17 on DVE)
     - k 0..5: 4x per-i tensor_scalar add+relu on DVE
     - 18 accumulating main matmuls rwr[k].T @ V[k] -> out psum [72, 384]
     Software-pipelined: build(g+1) is emitted before mains(g) so the PE
     never stalls on the relu copies.
Output per core: [72, 9216] with channel c = tag*24 + rel (rel_W pre-permuted
on host), reshaped on host to [3, 24, 96, 96].
"""
import sys

sys.path.insert(0, "/opt/trn_rl_repo")

import numpy as np

S, D, H3 = 96, 768, 2304
R, RSEQ, TAG, C = 24, 8, 3, 72
B = 8
KT = D // 128           # 6 d-chunks per half of proj_W
MT = H3 // 128          # 18 feature tiles
KD = 6                  # k-tiles 0..5 built DVE-direct
KP = MT - KD            # k-tiles 6..17 built via pair-matmul
PECOLS = KP * 128       # 1536 features in combined tiles
IGRP = 4
NG = S // IGRP          # 24 groups
NFREE = IGRP * S        # 384
NB = S // 32            # 3 combined blocks (32 i's each)
GPB = 32 // IGRP        # 8 groups per block
ACT_COPY_K = set(range(KD, KD + 10))   # pair-copy on ACT for k 6..15
SCALE = 1.0 / float(np.sqrt(np.float32(D)))


def build_nc(repeat: int = 1, debug: bool = False):
    import concourse.bass as bass
    from concourse import bacc, mybir
    import concourse.tile as tile
    from concourse.masks import make_identity

    f32 = mybir.dt.float32
    bf16 = mybir.dt.bfloat16
    AF = mybir.ActivationFunctionType
    ALU = mybir.AluOpType
    AX = mybir.AxisListType

    nc = bacc.Bacc()
    enc = nc.dram_tensor("enc", [S, D], f32, kind="ExternalInput")
    arel = nc.dram_tensor("arel", [RSEQ, D], f32, kind="ExternalInput")
    pw16 = nc.dram_tensor("pw16", [2 * D, H3], bf16, kind="ExternalInput")
    relw16 = nc.dram_tensor("relw16", [H3, C], bf16, kind="ExternalInput")
    selr_d = nc.dram_tensor("selr", [128, GPB * NFREE], bf16,
                            kind="ExternalInput")
    pbfm_d = nc.dram_tensor("pbfm", [128, MT], f32, kind="ExternalInput")
    pbnat_d = nc.dram_tensor("pbnat", [1, H3], bf16, kind="ExternalInput")
    out = nc.dram_tensor("out", [C, S * S], f32, kind="ExternalOutput")
    if debug:
        dbg_comb = nc.dram_tensor("dbg_comb", [128, PECOLS], f32,
                                  kind="ExternalOutput")
        dbg_v = nc.dram_tensor("dbg_v", [128, 3 * NFREE], f32,
                               kind="ExternalOutput")
        dbg_scan = nc.dram_tensor("dbg_scan", [RSEQ, S + 8], f32,
                                  kind="ExternalOutput")
        dbg_fm = nc.dram_tensor("dbg_fm", [128, 2 * S], f32,
                                kind="ExternalOutput")

    with tile.TileContext(nc) as tc:
        with (
            tc.tile_pool(name="persist", bufs=1) as pp,
            tc.tile_pool(name="work", bufs=2) as wp,
            tc.tile_pool(name="vpool", bufs=2) as vp,
            tc.tile_pool(name="scanp", bufs=2) as sp,
            tc.tile_pool(name="psmall", bufs=2, space="PSUM") as pss,
            tc.tile_pool(name="psone", bufs=1, space="PSUM") as ps1,
            tc.tile_pool(name="pspair", bufs=3, space="PSUM") as psq,
            tc.tile_pool(name="psout", bufs=2, space="PSUM") as pso,
        ):
            # ---------------- loads ----------------
            ident = pp.tile([128, 128], f32, tag="ident")
            make_identity(nc, ident[:])

            enc_nat = pp.tile([S, D], f32, tag="enc_nat")
            nc.sync.dma_start(enc_nat[:], enc[:])
            a_nat = pp.tile([RSEQ, D], f32, tag="a_nat")
            nc.sync.dma_start(a_nat[:], arel[:])
            selr = pp.tile([128, GPB * NFREE], bf16, tag="selr")
            nc.sync.dma_start(selr[:], selr_d[:])
            pbfm = pp.tile([128, MT], f32, tag="pbfm")
            nc.sync.dma_start(pbfm[:], pbfm_d[:])
            pbnat = pp.tile([1, H3], bf16, tag="pbnat")
            nc.sync.dma_start(pbnat[:], pbnat_d[:])
            rwr = []
            for k in range(MT):
                t = pp.tile([128, C], bf16, tag=f"rwr{k}")
                nc.sync.dma_start(t[:], relw16[k * 128:(k + 1) * 128, :])
                rwr.append(t)
            w16 = []
            for d in range(2 * KT):
                t = pp.tile([128, H3], bf16, tag=f"w16_{d}")
                nc.sync.dma_start(t[:], pw16[d * 128:(d + 1) * 128, :])
                w16.append(t)

            ones16 = pp.tile([1, S], bf16, tag="ones16")
            nc.vector.memset(ones16[:], 1.0)

            # ---------------- transposes / scan prep (f32) ----------------
            bT = []
            for k in range(KT):
                ps = pss.tile([128, S], f32, tag="tps")
                nc.tensor.transpose(
                    ps[:], enc_nat[:, k * 128:(k + 1) * 128], ident[:S, :S])
                t = pp.tile([128, S], f32, tag=f"bT{k}")
                nc.scalar.copy(t[:], ps[:])
                bT.append(t)
            at_raw, at_scl = [], []
            for k in range(KT):
                ps = pss.tile([128, RSEQ], f32, tag="tps")
                nc.tensor.transpose(
                    ps[:], a_nat[:, k * 128:(k + 1) * 128],
                    ident[:RSEQ, :RSEQ])
                tr = pp.tile([128, RSEQ], f32, tag=f"atr{k}")
                nc.scalar.copy(tr[:], ps[:])
                ts = pp.tile([128, RSEQ], f32, tag=f"ats{k}")
                nc.scalar.mul(ts[:], ps[:], SCALE)
                at_raw.append(tr)
                at_scl.append(ts)

            gps = pss.tile([RSEQ, RSEQ], f32, tag="tps")
            for k in range(KT):
                nc.tensor.matmul(gps[:], at_scl[k][:], at_raw[k][:],
                                 start=(k == 0), stop=(k == KT - 1))
            g_sb = pp.tile([RSEQ, RSEQ], f32, tag="g_sb")
            nc.vector.tensor_scalar_mul(g_sb[:], gps[:], 1.0)

            s_ps = ps1.tile([RSEQ, S], f32, tag="s_ps")
            for k in range(KT):
                nc.tensor.matmul(s_ps[:], at_scl[k][:], bT[k][:],
                                 start=(k == 0), stop=False,
                                 skip_group_check=True)

            # ---------------- scan (lagged max, G-folded normalizer) -------
            wsum = pp.tile([RSEQ, S], f32, tag="wsum")
            nc.vector.memset(wsum[:], 0.0)
            negmax = sp.tile([RSEQ, 1], f32, tag="negmax")
            nc.vector.reduce_max(negmax[:], s_ps[:], axis=AX.X, negate=True)
            for t in range(R):
                u = sp.tile([RSEQ, S], f32, tag="u")
                rs = sp.tile([RSEQ, 1], f32, tag="rs")
                nc.scalar.activation(u[:], s_ps[:], AF.Exp, bias=negmax[:],
                                     scale=1.0, accum_out=rs[:])
                rinv = sp.tile([RSEQ, 1], f32, tag="rinv")
                nc.vector.reciprocal(rinv[:], rs[:])
                # wsum accumulation (off critical chain)
                w = sp.tile([RSEQ, S], f32, tag="w")
                nc.vector.tensor_scalar_mul(w[:], u[:], rinv[:])
                nc.vector.tensor_tensor(wsum[:], wsum[:], w[:], op=ALU.add)
                if t < R - 1:
                    # lagged max: read s_t BEFORE the matmul updates it; the
                    # next exp sees s_{t+1} - max(s_t) <= ~30, safe in f32.
                    negmax = sp.tile([RSEQ, 1], f32, tag="negmax")
                    nc.vector.reduce_max(negmax[:], s_ps[:], axis=AX.X,
                                         negate=True)
                    gsc = sp.tile([RSEQ, RSEQ], f32, tag="gsc")
                    nc.vector.tensor_scalar_mul(gsc[:], g_sb[:], rinv[:])
                    nc.tensor.matmul(s_ps[:], gsc[:], u[:],
                                     start=False, stop=(t == R - 2),
                                     skip_group_check=True)

            # ---------------- b update + bf16 ----------------
            bT16 = []
            for k in range(KT):
                ps = pss.tile([128, S], f32, tag="tps")
                nc.tensor.matmul(ps[:], a_nat[:, k * 128:(k + 1) * 128],
                                 wsum[:], start=True, stop=True)
                nc.vector.tensor_tensor(bT[k][:], bT[k][:], ps[:], op=ALU.add)
                t16 = pp.tile([128, S], bf16, tag=f"bT16_{k}")
                nc.vector.tensor_scalar_mul(t16[:], bT[k][:], 1.0)
                bT16.append(t16)

            # ---------------- H projections ----------------
            # combined tiles: [0:96] = Ht_nat, [96:128] = Hh_nat rows 32b..
            # Both parts land in ONE psum tile per (block, chunk): the hh
            # rows are matmul'd straight into psum partitions 96..127 via
            # the auto-derived tile_position (M=32 at base 96).
            comb = []
            for b in range(NB):
                t = pp.tile([128, PECOLS], bf16, tag=f"comb{b}")
                comb.append(t)
            for b in range(NB):
                for cidx in range(3):
                    lo = cidx * 512
                    n = min(512, PECOLS - lo)
                    ps = pss.tile([128, 512], f32, tag="tps")
                    for d in range(KT):
                        nc.tensor.matmul(
                            ps[0:S, :n], bT16[d][:],
                            w16[KT + d][:, D + lo:D + lo + n],
                            start=(d == 0), stop=False,
                            skip_group_check=True)
                    for d in range(KT):
                        nc.tensor.matmul(
                            ps[S:128, :n], bT16[d][:, 32 * b:32 * b + 32],
                            w16[d][:, D + lo:D + lo + n],
                            start=(d == 0), stop=False,
                            skip_group_check=True, tile_position=(0, 96))
                    nc.tensor.matmul(ps[S:128, :n],
                                     ones16[:, 32 * b:32 * b + 32],
                                     pbnat[:, D + lo:D + lo + n],
                                     start=False, stop=True,
                                     skip_group_check=True,
                                     tile_position=(0, 96))
                    nc.scalar.activation(comb[b][:, lo:lo + n], ps[:, :n],
                                         AF.Identity, scale=1.0)

            # feature-major hh (f32 + pb) / ht (bf16) for k-tiles 0..5
            hh_fm, ht_fm = [], []
            for k in range(KD):
                msl = slice(k * 128, (k + 1) * 128)
                ps = pss.tile([128, S], f32, tag="tps")
                for d in range(KT):
                    nc.tensor.matmul(ps[:], w16[d][:, msl], bT16[d][:],
                                     start=(d == 0), stop=(d == KT - 1))
                th = pp.tile([128, S], f32, tag=f"hhfm{k}")
                nc.scalar.activation(th[:], ps[:], AF.Identity,
                                     bias=pbfm[:, k:k + 1], scale=1.0)
                hh_fm.append(th)
                ps2 = pss.tile([128, S], f32, tag="tps")
                for d in range(KT):
                    nc.tensor.matmul(ps2[:], w16[KT + d][:, msl], bT16[d][:],
                                     start=(d == 0), stop=(d == KT - 1))
                tt = pp.tile([128, S], bf16, tag=f"htfm{k}")
                nc.vector.tensor_scalar_mul(tt[:], ps2[:], 1.0)
                ht_fm.append(tt)

            # ---------------- main loop (software-pipelined) ---------------
            def build(g):
                b, gb = g // GPB, g % GPB
                vt = {}
                for k in range(KD, MT):
                    pq = psq.tile([128, NFREE], f32, tag="pairps")
                    nc.tensor.matmul(
                        pq[:], comb[b][:, (k - KD) * 128:(k - KD + 1) * 128],
                        selr[:, gb * NFREE:(gb + 1) * NFREE],
                        start=True, stop=True)
                    v = vp.tile([128, NFREE], bf16, tag=f"v{k}")
                    if k in ACT_COPY_K:
                        nc.scalar.activation(v[:], pq[:], AF.Relu, scale=1.0)
                    else:
                        nc.vector.tensor_scalar_max(v[:], pq[:], 0.0)
                    vt[k] = v
                for k in range(KD):
                    v = vp.tile([128, NFREE], bf16, tag=f"v{k}")
                    for ii in range(IGRP):
                        i = g * IGRP + ii
                        nc.vector.tensor_scalar(
                            v[:, ii * S:(ii + 1) * S], ht_fm[k][:],
                            hh_fm[k][:, i:i + 1], 0.0,
                            op0=ALU.add, op1=ALU.max)
                    vt[k] = v
                return vt

            def mains(g, vt):
                ops = pso.tile([C, NFREE], f32, tag="ops")
                for j, k in enumerate(range(MT)):
                    nc.tensor.matmul(ops[:], rwr[k][:], vt[k][:],
                                     start=(j == 0), stop=(j == MT - 1))
                ostg = wp.tile([C, NFREE], f32, tag="ostg")
                if g % 2 == 0:
                    nc.scalar.activation(ostg[:], ops[:], AF.Identity,
                                         scale=1.0)
                else:
                    nc.vector.tensor_scalar_mul(ostg[:], ops[:], 1.0)
                nc.sync.dma_start(out[:, g * NFREE:(g + 1) * NFREE], ostg[:])

            prev = build(0)
            if debug:
                stg = wp.tile([128, 3 * NFREE], f32, tag="dbgv")
                nc.vector.tensor_scalar_mul(stg[:, :NFREE], prev[0][:], 1.0)
                nc.vector.tensor_scalar_mul(
                    stg[:, NFREE:2 * NFREE], prev[6][:], 1.0)
                nc.vector.tensor_scalar_mul(
                    stg[:, 2 * NFREE:], prev[17][:], 1.0)
                nc.sync.dma_start(dbg_v[:], stg[:])
                stg2 = wp.tile([128, PECOLS], f32, tag="dbgc")
                nc.vector.tensor_scalar_mul(stg2[:], comb[0][:], 1.0)
                nc.sync.dma_start(dbg_comb[:], stg2[:])
                stg3 = wp.tile([RSEQ, S + 8], f32, tag="dbgs")
                nc.vector.tensor_scalar_mul(stg3[:, :S], wsum[:], 1.0)
                nc.vector.tensor_scalar_mul(stg3[:, S:], g_sb[:], 1.0)
                nc.sync.dma_start(dbg_scan[:], stg3[:])
                stg4 = wp.tile([128, 2 * S], f32, tag="dbgf")
                nc.vector.tensor_scalar_mul(stg4[:, :S], hh_fm[0][:], 1.0)
                nc.vector.tensor_scalar_mul(stg4[:, S:], ht_fm[0][:], 1.0)
                nc.sync.dma_start(dbg_fm[:], stg4[:])
            for g in range(1, NG):
                cur = build(g)
                mains(g - 1, prev)
                prev = cur
            mains(NG - 1, prev)

    nc.finalize()
    return nc


_CACHED_NC = None


def _host_consts():
    import ml_dtypes
    bf = ml_dtypes.bfloat16
    sel = np.zeros((128, GPB * NFREE), np.float32)
    for gb in range(GPB):
        base = gb * NFREE
        for ii in range(IGRP):
            sel[np.arange(S), base + ii * S + np.arange(S)] = 1.0
            sel[S + gb * IGRP + ii, base + ii * S:base + (ii + 1) * S] = 1.0
    return sel.astype(bf)


def _prep_in_maps(encoded_text, rel_types_encoded, proj_W, proj_b, rel_W):
    import ml_dtypes
    bf = ml_dtypes.bfloat16
    relw_perm = np.ascontiguousarray(
        rel_W.reshape(H3, R, TAG).transpose(0, 2, 1).reshape(H3, C)
    ).astype(bf)
    pw16 = np.ascontiguousarray(proj_W).astype(bf)
    selr = _host_consts()
    pb32 = np.asarray(proj_b, dtype=np.float32)
    pbfm = np.ascontiguousarray(pb32.reshape(MT, 128).T)  # [128, MT]
    pbnat = pb32.reshape(1, H3).astype(bf)
    in_maps = []
    for i in range(B):
        in_maps.append({
            "enc": np.ascontiguousarray(encoded_text[i], dtype=np.float32),
            "arel": np.ascontiguousarray(
                rel_types_encoded[i], dtype=np.float32),
            "pw16": pw16,
            "relw16": relw_perm,
            "selr": selr,
            "pbfm": pbfm,
            "pbnat": pbnat,
        })
    return in_maps


def _assemble(results, rel_b):
    outs = []
    for i in range(B):
        o = results[i]["out"].reshape(TAG, R, S, S)
        outs.append(o)
    full = np.stack(outs, axis=0).astype(np.float32)
    if np.any(rel_b):
        relb_perm = np.asarray(rel_b, dtype=np.float32).reshape(R, TAG).T
        full = full + relb_perm[None, :, :, None, None]
    return full


def kernel(encoded_text, rel_types_encoded, proj_W, proj_b, rel_W, rel_b):
    global _CACHED_NC
    from concourse.bass_utils import run_bass_kernel_spmd

    if _CACHED_NC is None:
        _CACHED_NC = build_nc()
    in_maps = _prep_in_maps(
        encoded_text, rel_types_encoded, proj_W, proj_b, rel_W)
    res = run_bass_kernel_spmd(_CACHED_NC, in_maps, list(range(B)))
    return _assemble(res.results, rel_b)


# revision 11
# speedup vs baseline: 1.3400x; 1.0651x over previous
"""Trainium2 Bass kernel for nn_AttModel (B=8, S=96, D=768, R=24, RSEQ=8, TAG=3).

Data-parallel over batch: core i handles sample i. v2 design:

  1. Host pre-converts proj_W / rel_W(permuted) / pair-selectors to bf16;
     W DMA is ~21us instead of 42us.
  2. Refine scan in score space with LAGGED max (reduce_max runs off the
     critical chain; exp(s_t - max(s_{t-1})) <= e^30, safe in f32) and the
     softmax normalizer folded into the tiny [8,8] G-scale instead of a
     [8,96] w-scale.
  3. H projections in bf16, two layouts:
     - feature-major hh/ht [128, 96] for k-tiles 0..5 (DVE-direct V build)
     - natural-layout combined tiles [128, 1536] for k-tiles 6..17:
       partitions 0..95 = Ht_nat(j), 96..127 = Hh_nat rows 32b..32b+31
  4. Main loop per group g (4 i's x 96 j's = 384 pairs):
     - k 6..17: pair-matmul  P = combined[:, kslice].T @ selR[g%8]  on PE,
       then relu PSUM->SBUF bf16 copy (k 6..15 on ACT, 16..17 on DVE)
     - k 0..5: 4x per-i tensor_scalar add+relu on DVE
     - 18 accumulating main matmuls rwr[k].T @ V[k] -> out psum [72, 384]
     Software-pipelined: build(g+1) is emitted before mains(g) so the PE
     never stalls on the relu copies.
Output per core: [72, 9216] with channel c = tag*24 + rel (rel_W pre-permuted
on host), reshaped on host to [3, 24, 96, 96].
"""
import sys

sys.path.insert(0, "/opt/trn_rl_repo")

import numpy as np

S, D, H3 = 96, 768, 2304
R, RSEQ, TAG, C = 24, 8, 3, 72
B = 8
KT = D // 128           # 6 d-chunks per half of proj_W
MT = H3 // 128          # 18 feature tiles
KD = 6                  # k-tiles 0..5 built DVE-direct
KP = MT - KD            # k-tiles 6..17 built via pair-matmul
PECOLS = KP * 128       # 1536 features in combined tiles
IGRP = 4
NG = S // IGRP          # 24 groups
NFREE = IGRP * S        # 384
NB = S // 32            # 3 combined blocks (32 i's each)
GPB = 32 // IGRP        # 8 groups per block
ACT_COPY_K = set(range(KD, KD + 10))   # pair-copy on ACT for k 6..15
SCALE = 1.0 / float(np.sqrt(np.float32(D)))


def build_nc(repeat: int = 1, debug: bool = False):
    import concourse.bass as bass
    from concourse import bacc, mybir
    import concourse.tile as tile
    from concourse.masks import make_identity

    f32 = mybir.dt.float32
    bf16 = mybir.dt.bfloat16
    AF = mybir.ActivationFunctionType
    ALU = mybir.AluOpType
    AX = mybir.AxisListType

    nc = bacc.Bacc()
    enc = nc.dram_tensor("enc", [S, D], f32, kind="ExternalInput")
    arel = nc.dram_tensor("arel", [RSEQ, D], f32, kind="ExternalInput")
    pw16 = nc.dram_tensor("pw16", [2 * D, H3], bf16, kind="ExternalInput")
    relw16 = nc.dram_tensor("relw16", [H3, C], bf16, kind="ExternalInput")
    selr_d = nc.dram_tensor("selr", [128, GPB * NFREE], bf16,
                            kind="ExternalInput")
    pbfm_d = nc.dram_tensor("pbfm", [128, MT], f32, kind="ExternalInput")
    pbnat_d = nc.dram_tensor("pbnat", [1, H3], bf16, kind="ExternalInput")
    out = nc.dram_tensor("out", [C, S * S], f32, kind="ExternalOutput")
    if debug:
        dbg_comb = nc.dram_tensor("dbg_comb", [128, PECOLS], f32,
                                  kind="ExternalOutput")
        dbg_v = nc.dram_tensor("dbg_v", [128, 3 * NFREE], f32,
                               kind="ExternalOutput")
        dbg_scan = nc.dram_tensor("dbg_scan", [RSEQ, S + 8], f32,
                                  kind="ExternalOutput")
        dbg_fm = nc.dram_tensor("dbg_fm", [128, 2 * S], f32,
                                kind="ExternalOutput")

    with tile.TileContext(nc) as tc:
        with (
            tc.tile_pool(name="persist", bufs=1) as pp,
            tc.tile_pool(name="work", bufs=2) as wp,
            tc.tile_pool(name="vpool", bufs=3) as vp,
            tc.tile_pool(name="scanp", bufs=2) as sp,
            tc.tile_pool(name="psmall", bufs=2, space="PSUM") as pss,
            tc.tile_pool(name="psone", bufs=1, space="PSUM") as ps1,
            tc.tile_pool(name="pspair", bufs=3, space="PSUM") as psq,
            tc.tile_pool(name="psout", bufs=2, space="PSUM") as pso,
        ):
            # ---------------- loads ----------------
            ident = pp.tile([128, 128], f32, tag="ident")
            make_identity(nc, ident[:])

            enc_nat = pp.tile([S, D], f32, tag="enc_nat")
            nc.sync.dma_start(enc_nat[:], enc[:])
            a_nat = pp.tile([RSEQ, D], f32, tag="a_nat")
            nc.sync.dma_start(a_nat[:], arel[:])
            selr = pp.tile([128, GPB * NFREE], bf16, tag="selr")
            nc.sync.dma_start(selr[:], selr_d[:])
            pbfm = pp.tile([128, MT], f32, tag="pbfm")
            nc.sync.dma_start(pbfm[:], pbfm_d[:])
            pbnat = pp.tile([1, H3], bf16, tag="pbnat")
            nc.sync.dma_start(pbnat[:], pbnat_d[:])
            rwr = []
            for k in range(MT):
                t = pp.tile([128, C], bf16, tag=f"rwr{k}")
                nc.sync.dma_start(t[:], relw16[k * 128:(k + 1) * 128, :])
                rwr.append(t)
            w16 = []
            for d in range(2 * KT):
                t = pp.tile([128, H3], bf16, tag=f"w16_{d}")
                nc.sync.dma_start(t[:], pw16[d * 128:(d + 1) * 128, :])
                w16.append(t)

            ones16 = pp.tile([1, S], bf16, tag="ones16")
            nc.vector.memset(ones16[:], 1.0)

            # ---------------- transposes / scan prep (f32) ----------------
            bT = []
            for k in range(KT):
                ps = pss.tile([128, S], f32, tag="tps")
                nc.tensor.transpose(
                    ps[:], enc_nat[:, k * 128:(k + 1) * 128], ident[:S, :S])
                t = pp.tile([128, S], f32, tag=f"bT{k}")
                nc.scalar.copy(t[:], ps[:])
                bT.append(t)
            at_raw, at_scl = [], []
            for k in range(KT):
                ps = pss.tile([128, RSEQ], f32, tag="tps")
                nc.tensor.transpose(
                    ps[:], a_nat[:, k * 128:(k + 1) * 128],
                    ident[:RSEQ, :RSEQ])
                tr = pp.tile([128, RSEQ], f32, tag=f"atr{k}")
                nc.scalar.copy(tr[:], ps[:])
                ts = pp.tile([128, RSEQ], f32, tag=f"ats{k}")
                nc.scalar.mul(ts[:], ps[:], SCALE)
                at_raw.append(tr)
                at_scl.append(ts)

            gps = pss.tile([RSEQ, RSEQ], f32, tag="tps")
            for k in range(KT):
                nc.tensor.matmul(gps[:], at_scl[k][:], at_raw[k][:],
                                 start=(k == 0), stop=(k == KT - 1))
            g_sb = pp.tile([RSEQ, RSEQ], f32, tag="g_sb")
            nc.vector.tensor_scalar_mul(g_sb[:], gps[:], 1.0)

            s_ps = ps1.tile([RSEQ, S], f32, tag="s_ps")
            for k in range(KT):
                nc.tensor.matmul(s_ps[:], at_scl[k][:], bT[k][:],
                                 start=(k == 0), stop=False,
                                 skip_group_check=True)

            # ---------------- scan (lagged max, G-folded normalizer) -------
            wsum = pp.tile([RSEQ, S], f32, tag="wsum")
            nc.vector.memset(wsum[:], 0.0)
            negmax = sp.tile([RSEQ, 1], f32, tag="negmax")
            nc.vector.reduce_max(negmax[:], s_ps[:], axis=AX.X, negate=True)
            for t in range(R):
                u = sp.tile([RSEQ, S], f32, tag="u")
                rs = sp.tile([RSEQ, 1], f32, tag="rs")
                nc.scalar.activation(u[:], s_ps[:], AF.Exp, bias=negmax[:],
                                     scale=1.0, accum_out=rs[:])
                rinv = sp.tile([RSEQ, 1], f32, tag="rinv")
                nc.vector.reciprocal(rinv[:], rs[:])
                # wsum accumulation (off critical chain)
                w = sp.tile([RSEQ, S], f32, tag="w")
                nc.vector.tensor_scalar_mul(w[:], u[:], rinv[:])
                nc.vector.tensor_tensor(wsum[:], wsum[:], w[:], op=ALU.add)
                if t < R - 1:
                    # lagged max: read s_t BEFORE the matmul updates it; the
                    # next exp sees s_{t+1} - max(s_t) <= ~30, safe in f32.
                    negmax = sp.tile([RSEQ, 1], f32, tag="negmax")
                    nc.vector.reduce_max(negmax[:], s_ps[:], axis=AX.X,
                                         negate=True)
                    gsc = sp.tile([RSEQ, RSEQ], f32, tag="gsc")
                    nc.vector.tensor_scalar_mul(gsc[:], g_sb[:], rinv[:])
                    nc.tensor.matmul(s_ps[:], gsc[:], u[:],
                                     start=False, stop=(t == R - 2),
                                     skip_group_check=True)

            # ---------------- b update + bf16 ----------------
            bT16 = []
            for k in range(KT):
                ps = pss.tile([128, S], f32, tag="tps")
                nc.tensor.matmul(ps[:], a_nat[:, k * 128:(k + 1) * 128],
                                 wsum[:], start=True, stop=True)
                nc.vector.tensor_tensor(bT[k][:], bT[k][:], ps[:], op=ALU.add)
                t16 = pp.tile([128, S], bf16, tag=f"bT16_{k}")
                nc.vector.tensor_scalar_mul(t16[:], bT[k][:], 1.0)
                bT16.append(t16)

            # ---------------- H projections ----------------
            # combined tiles: [0:96] = Ht_nat, [96:128] = Hh_nat rows 32b..
            # Both parts land in ONE psum tile per (block, chunk): the hh
            # rows are matmul'd straight into psum partitions 96..127 via
            # the auto-derived tile_position (M=32 at base 96).
            comb = []
            for b in range(NB):
                t = pp.tile([128, PECOLS], bf16, tag=f"comb{b}")
                comb.append(t)
            def comb_unit(b, cidx):
                lo = cidx * 512
                n = min(512, PECOLS - lo)
                ps = pss.tile([128, 512], f32, tag="tps")
                for d in range(KT):
                    nc.tensor.matmul(
                        ps[0:S, :n], bT16[d][:],
                        w16[KT + d][:, D + lo:D + lo + n],
                        start=(d == 0), stop=False,
                        skip_group_check=True)
                for d in range(KT):
                    nc.tensor.matmul(
                        ps[S:128, :n], bT16[d][:, 32 * b:32 * b + 32],
                        w16[d][:, D + lo:D + lo + n],
                        start=(d == 0), stop=False,
                        skip_group_check=True, tile_position=(0, 96))
                nc.tensor.matmul(ps[S:128, :n],
                                 ones16[:, 32 * b:32 * b + 32],
                                 pbnat[:, D + lo:D + lo + n],
                                 start=False, stop=True,
                                 skip_group_check=True,
                                 tile_position=(0, 96))
                nc.scalar.activation(comb[b][:, lo:lo + n], ps[:, :n],
                                     AF.Identity, scale=1.0)

            for cidx in range(3):
                comb_unit(0, cidx)

            # feature-major hh (f32 + pb) / ht (bf16) for k-tiles 0..5
            hh_fm, ht_fm = [], []
            for k in range(KD):
                msl = slice(k * 128, (k + 1) * 128)
                ps = pss.tile([128, S], f32, tag="tps")
                for d in range(KT):
                    nc.tensor.matmul(ps[:], w16[d][:, msl], bT16[d][:],
                                     start=(d == 0), stop=(d == KT - 1))
                th = pp.tile([128, S], f32, tag=f"hhfm{k}")
                nc.scalar.activation(th[:], ps[:], AF.Identity,
                                     bias=pbfm[:, k:k + 1], scale=1.0)
                hh_fm.append(th)
                ps2 = pss.tile([128, S], f32, tag="tps")
                for d in range(KT):
                    nc.tensor.matmul(ps2[:], w16[KT + d][:, msl], bT16[d][:],
                                     start=(d == 0), stop=(d == KT - 1))
                tt = pp.tile([128, S], bf16, tag=f"htfm{k}")
                nc.vector.tensor_scalar_mul(tt[:], ps2[:], 1.0)
                ht_fm.append(tt)

            # ---------------- main loop (software-pipelined) ---------------
            def build(g):
                b, gb = g // GPB, g % GPB
                vt = {}
                for k in range(KD, MT):
                    if (k - KD) % 5 < 3:
                        pq = psq.tile([128, NFREE], f32, tag="pairps")
                    else:
                        pq = pss.tile([128, NFREE], f32, tag="tps")
                    nc.tensor.matmul(
                        pq[:], comb[b][:, (k - KD) * 128:(k - KD + 1) * 128],
                        selr[:, gb * NFREE:(gb + 1) * NFREE],
                        start=True, stop=True)
                    v = vp.tile([128, NFREE], bf16, tag=f"v{k}")
                    if k in ACT_COPY_K:
                        nc.scalar.activation(v[:], pq[:], AF.Relu, scale=1.0)
                    else:
                        nc.vector.tensor_scalar_max(v[:], pq[:], 0.0)
                    vt[k] = v
                for k in range(KD):
                    v = vp.tile([128, NFREE], bf16, tag=f"v{k}")
                    for ii in range(IGRP):
                        i = g * IGRP + ii
                        nc.vector.tensor_scalar(
                            v[:, ii * S:(ii + 1) * S], ht_fm[k][:],
                            hh_fm[k][:, i:i + 1], 0.0,
                            op0=ALU.add, op1=ALU.max)
                    vt[k] = v
                return vt

            def mains(g, vt):
                ops = pso.tile([C, NFREE], f32, tag="ops")
                for j, k in enumerate(range(MT)):
                    nc.tensor.matmul(ops[:], rwr[k][:], vt[k][:],
                                     start=(j == 0), stop=(j == MT - 1))
                ostg = wp.tile([C, NFREE], f32, tag="ostg")
                if g % 2 == 0:
                    nc.scalar.activation(ostg[:], ops[:], AF.Identity,
                                         scale=1.0)
                else:
                    nc.vector.tensor_scalar_mul(ostg[:], ops[:], 1.0)
                nc.sync.dma_start(out[:, g * NFREE:(g + 1) * NFREE], ostg[:])

            prev = build(0)
            if debug:
                stg = wp.tile([128, 3 * NFREE], f32, tag="dbgv")
                nc.vector.tensor_scalar_mul(stg[:, :NFREE], prev[0][:], 1.0)
                nc.vector.tensor_scalar_mul(
                    stg[:, NFREE:2 * NFREE], prev[6][:], 1.0)
                nc.vector.tensor_scalar_mul(
                    stg[:, 2 * NFREE:], prev[17][:], 1.0)
                nc.sync.dma_start(dbg_v[:], stg[:])
                stg2 = wp.tile([128, PECOLS], f32, tag="dbgc")
                nc.vector.tensor_scalar_mul(stg2[:], comb[0][:], 1.0)
                nc.sync.dma_start(dbg_comb[:], stg2[:])
                stg3 = wp.tile([RSEQ, S + 8], f32, tag="dbgs")
                nc.vector.tensor_scalar_mul(stg3[:, :S], wsum[:], 1.0)
                nc.vector.tensor_scalar_mul(stg3[:, S:], g_sb[:], 1.0)
                nc.sync.dma_start(dbg_scan[:], stg3[:])
                stg4 = wp.tile([128, 2 * S], f32, tag="dbgf")
                nc.vector.tensor_scalar_mul(stg4[:, :S], hh_fm[0][:], 1.0)
                nc.vector.tensor_scalar_mul(stg4[:, S:], ht_fm[0][:], 1.0)
                nc.sync.dma_start(dbg_fm[:], stg4[:])
            for g in range(1, NG):
                if g in (2, 4, 6) or g in (10, 12, 14):
                    comb_unit(1 if g < 8 else 2, (g % 8) // 2 - 1)
                cur = build(g)
                mains(g - 1, prev)
                prev = cur
            mains(NG - 1, prev)

    nc.finalize()
    return nc


_CACHED_NC = None


def _host_consts():
    import ml_dtypes
    bf = ml_dtypes.bfloat16
    sel = np.zeros((128, GPB * NFREE), np.float32)
    for gb in range(GPB):
        base = gb * NFREE
        for ii in range(IGRP):
            sel[np.arange(S), base + ii * S + np.arange(S)] = 1.0
            sel[S + gb * IGRP + ii, base + ii * S:base + (ii + 1) * S] = 1.0
    return sel.astype(bf)


def _prep_in_maps(encoded_text, rel_types_encoded, proj_W, proj_b, rel_W):
    import ml_dtypes
    bf = ml_dtypes.bfloat16
    relw_perm = np.ascontiguousarray(
        rel_W.reshape(H3, R, TAG).transpose(0, 2, 1).reshape(H3, C)
    ).astype(bf)
    pw16 = np.ascontiguousarray(proj_W).astype(bf)
    selr = _host_consts()
    pb32 = np.asarray(proj_b, dtype=np.float32)
    pbfm = np.ascontiguousarray(pb32.reshape(MT, 128).T)  # [128, MT]
    pbnat = pb32.reshape(1, H3).astype(bf)
    in_maps = []
    for i in range(B):
        in_maps.append({
            "enc": np.ascontiguousarray(encoded_text[i], dtype=np.float32),
            "arel": np.ascontiguousarray(
                rel_types_encoded[i], dtype=np.float32),
            "pw16": pw16,
            "relw16": relw_perm,
            "selr": selr,
            "pbfm": pbfm,
            "pbnat": pbnat,
        })
    return in_maps


def _assemble(results, rel_b):
    outs = []
    for i in range(B):
        o = results[i]["out"].reshape(TAG, R, S, S)
        outs.append(o)
    full = np.stack(outs, axis=0).astype(np.float32)
    if np.any(rel_b):
        relb_perm = np.asarray(rel_b, dtype=np.float32).reshape(R, TAG).T
        full = full + relb_perm[None, :, :, None, None]
    return full


def kernel(encoded_text, rel_types_encoded, proj_W, proj_b, rel_W, rel_b):
    global _CACHED_NC
    from concourse.bass_utils import run_bass_kernel_spmd

    if _CACHED_NC is None:
        _CACHED_NC = build_nc()
    in_maps = _prep_in_maps(
        encoded_text, rel_types_encoded, proj_W, proj_b, rel_W)
    res = run_bass_kernel_spmd(_CACHED_NC, in_maps, list(range(B)))
    return _assemble(res.results, rel_b)


# revision 15
# speedup vs baseline: 1.3737x; 1.0251x over previous
"""Trainium2 Bass kernel for nn_AttModel (B=8, S=96, D=768, R=24, RSEQ=8, TAG=3).

Data-parallel over batch: core i handles sample i. v2 design:

  1. Host pre-converts proj_W / rel_W(permuted) / pair-selectors to bf16;
     W DMA is ~21us instead of 42us.
  2. Refine scan in score space with LAGGED max (reduce_max runs off the
     critical chain; exp(s_t - max(s_{t-1})) <= e^30, safe in f32) and the
     softmax normalizer folded into the tiny [8,8] G-scale instead of a
     [8,96] w-scale.
  3. H projections in bf16, two layouts:
     - feature-major hh/ht [128, 96] for k-tiles 0..5 (DVE-direct V build)
     - natural-layout combined tiles [128, 1536] for k-tiles 6..17:
       partitions 0..95 = Ht_nat(j), 96..127 = Hh_nat rows 32b..32b+31
  4. Main loop per group g (4 i's x 96 j's = 384 pairs):
     - k 6..17: pair-matmul  P = combined[:, kslice].T @ selR[g%8]  on PE,
       then relu PSUM->SBUF bf16 copy (k 6..15 on ACT, 16..17 on DVE)
     - k 0..5: 4x per-i tensor_scalar add+relu on DVE
     - 18 accumulating main matmuls rwr[k].T @ V[k] -> out psum [72, 384]
     Software-pipelined: build(g+1) is emitted before mains(g) so the PE
     never stalls on the relu copies.
Output per core: [72, 9216] with channel c = tag*24 + rel (rel_W pre-permuted
on host), reshaped on host to [3, 24, 96, 96].
"""
import sys

sys.path.insert(0, "/opt/trn_rl_repo")

import numpy as np

S, D, H3 = 96, 768, 2304
R, RSEQ, TAG, C = 24, 8, 3, 72
B = 8
KT = D // 128           # 6 d-chunks per half of proj_W
MT = H3 // 128          # 18 feature tiles
KD = 7                  # k-tiles 0..KD-1 built DVE-direct
KP = MT - KD            # k-tiles 6..17 built via pair-matmul
PECOLS = KP * 128       # 1536 features in combined tiles
IGRP = 4
NG = S // IGRP          # 24 groups
NFREE = IGRP * S        # 384
NB = S // 32            # 3 combined blocks (32 i's each)
GPB = 32 // IGRP        # 8 groups per block
ACT_COPY_K = set(range(KD, KD + 10))   # pair-copy on ACT; rest on DVE
SCALE = 1.0 / float(np.sqrt(np.float32(D)))


def build_nc(repeat: int = 1, debug: bool = False):
    import concourse.bass as bass
    from concourse import bacc, mybir
    import concourse.tile as tile
    from concourse.masks import make_identity

    f32 = mybir.dt.float32
    bf16 = mybir.dt.bfloat16
    AF = mybir.ActivationFunctionType
    ALU = mybir.AluOpType
    AX = mybir.AxisListType

    nc = bacc.Bacc()
    enc = nc.dram_tensor("enc", [S, D], f32, kind="ExternalInput")
    arel = nc.dram_tensor("arel", [RSEQ, D], f32, kind="ExternalInput")
    pw16 = nc.dram_tensor("pw16", [2 * D, H3], bf16, kind="ExternalInput")
    relw16 = nc.dram_tensor("relw16", [H3, C], bf16, kind="ExternalInput")
    selr_d = nc.dram_tensor("selr", [128, GPB * NFREE], bf16,
                            kind="ExternalInput")
    pbfm_d = nc.dram_tensor("pbfm", [128, MT], f32, kind="ExternalInput")
    pbnat_d = nc.dram_tensor("pbnat", [1, H3], bf16, kind="ExternalInput")
    out = nc.dram_tensor("out", [C, S * S], f32, kind="ExternalOutput")
    if debug:
        dbg_comb = nc.dram_tensor("dbg_comb", [128, PECOLS], f32,
                                  kind="ExternalOutput")
        dbg_v = nc.dram_tensor("dbg_v", [128, 3 * NFREE], f32,
                               kind="ExternalOutput")
        dbg_scan = nc.dram_tensor("dbg_scan", [RSEQ, S + 8], f32,
                                  kind="ExternalOutput")
        dbg_fm = nc.dram_tensor("dbg_fm", [128, 2 * S], f32,
                                kind="ExternalOutput")

    with tile.TileContext(nc) as tc:
        with (
            tc.tile_pool(name="persist", bufs=1) as pp,
            tc.tile_pool(name="work", bufs=2) as wp,
            tc.tile_pool(name="vpool", bufs=3) as vp,
            tc.tile_pool(name="scanp", bufs=2) as sp,
            tc.tile_pool(name="psmall", bufs=2, space="PSUM") as pss,
            tc.tile_pool(name="psone", bufs=1, space="PSUM") as ps1,
            tc.tile_pool(name="pspair", bufs=3, space="PSUM") as psq,
            tc.tile_pool(name="psout", bufs=2, space="PSUM") as pso,
        ):
            # ---------------- loads ----------------
            ident = pp.tile([128, 128], f32, tag="ident")
            make_identity(nc, ident[:])

            enc_nat = pp.tile([S, D], f32, tag="enc_nat")
            nc.sync.dma_start(enc_nat[:], enc[:])
            a_nat = pp.tile([RSEQ, D], f32, tag="a_nat")
            nc.sync.dma_start(a_nat[:], arel[:])
            selr = pp.tile([128, GPB * NFREE], bf16, tag="selr")
            nc.sync.dma_start(selr[:], selr_d[:])
            pbfm = pp.tile([128, MT], f32, tag="pbfm")
            nc.sync.dma_start(pbfm[:], pbfm_d[:])
            pbnat = pp.tile([1, H3], bf16, tag="pbnat")
            nc.sync.dma_start(pbnat[:], pbnat_d[:])
            rwrb = pp.tile([128, MT * C], bf16, tag="rwrb")
            nc.sync.dma_start(
                rwrb[:].rearrange("p (k c) -> p k c", k=MT),
                relw16.rearrange("(k p) c -> p k c", p=128))
            rwr = [rwrb[:, k * C:(k + 1) * C] for k in range(MT)]
            wb = pp.tile([128, 2 * KT * H3], bf16, tag="wb")
            nc.sync.dma_start(
                wb[:].rearrange("p (n m) -> p n m", n=2 * KT),
                pw16.rearrange("(n p) m -> p n m", p=128))
            w16 = [wb[:, d * H3:(d + 1) * H3] for d in range(2 * KT)]

            ones16 = pp.tile([1, S], bf16, tag="ones16")
            nc.vector.memset(ones16[:], 1.0)

            # ---------------- transposes / scan prep (f32) ----------------
            bT = []
            for k in range(KT):
                ps = pss.tile([128, S], f32, tag="tps")
                nc.tensor.transpose(
                    ps[:], enc_nat[:, k * 128:(k + 1) * 128], ident[:S, :S])
                t = pp.tile([128, S], f32, tag=f"bT{k}")
                nc.scalar.copy(t[:], ps[:])
                bT.append(t)
            at_raw, at_scl = [], []
            for k in range(KT):
                ps = pss.tile([128, RSEQ], f32, tag="tps")
                nc.tensor.transpose(
                    ps[:], a_nat[:, k * 128:(k + 1) * 128],
                    ident[:RSEQ, :RSEQ])
                tr = pp.tile([128, RSEQ], f32, tag=f"atr{k}")
                nc.scalar.copy(tr[:], ps[:])
                ts = pp.tile([128, RSEQ], f32, tag=f"ats{k}")
                nc.scalar.mul(ts[:], ps[:], SCALE)
                at_raw.append(tr)
                at_scl.append(ts)

            gps = pss.tile([RSEQ, RSEQ], f32, tag="tps")
            for k in range(KT):
                nc.tensor.matmul(gps[:], at_scl[k][:], at_raw[k][:],
                                 start=(k == 0), stop=(k == KT - 1))
            g_sb = pp.tile([RSEQ, RSEQ], f32, tag="g_sb")
            nc.vector.tensor_scalar_mul(g_sb[:], gps[:], 1.0)

            s_ps = ps1.tile([RSEQ, S], f32, tag="s_ps")
            for k in range(KT):
                nc.tensor.matmul(s_ps[:], at_scl[k][:], bT[k][:],
                                 start=(k == 0), stop=False,
                                 skip_group_check=True)

            # ---------------- scan (lagged max, G-folded normalizer) -------
            wsum = pp.tile([RSEQ, S], f32, tag="wsum")
            nc.vector.memset(wsum[:], 0.0)
            negmax = sp.tile([RSEQ, 1], f32, tag="negmax")
            nc.vector.reduce_max(negmax[:], s_ps[:], axis=AX.X, negate=True)
            for t in range(R):
                u = sp.tile([RSEQ, S], f32, tag="u")
                rs = sp.tile([RSEQ, 1], f32, tag="rs")
                nc.scalar.activation(u[:], s_ps[:], AF.Exp, bias=negmax[:],
                                     scale=1.0, accum_out=rs[:])
                rinv = sp.tile([RSEQ, 1], f32, tag="rinv")
                nc.vector.reciprocal(rinv[:], rs[:])
                if t < R - 1:
                    # critical chain: gsc = G * rinv, then the matmul.
                    gsc = sp.tile([RSEQ, RSEQ], f32, tag="gsc")
                    nc.vector.tensor_scalar_mul(gsc[:], g_sb[:], rinv[:])
                    # lagged max: read s_t BEFORE the matmul updates it; the
                    # next exp sees s_{t+1} - max(s_t) <= ~30, safe in f32.
                    negmax = sp.tile([RSEQ, 1], f32, tag="negmax")
                    nc.vector.reduce_max(negmax[:], s_ps[:], axis=AX.X,
                                         negate=True)
                    nc.tensor.matmul(s_ps[:], gsc[:], u[:],
                                     start=False, stop=(t == R - 2),
                                     skip_group_check=True)
                # wsum += u * rinv (off critical chain)
                nc.vector.scalar_tensor_tensor(
                    wsum[:], u[:], rinv[:], wsum[:],
                    op0=ALU.mult, op1=ALU.add)

            # ---------------- b update + bf16 ----------------
            bT16 = []
            for k in range(KT):
                ps = pss.tile([128, S], f32, tag="tps")
                nc.tensor.matmul(ps[:], a_nat[:, k * 128:(k + 1) * 128],
                                 wsum[:], start=True, stop=True)
                nc.vector.tensor_tensor(bT[k][:], bT[k][:], ps[:], op=ALU.add)
                t16 = pp.tile([128, S], bf16, tag=f"bT16_{k}")
                nc.vector.tensor_scalar_mul(t16[:], bT[k][:], 1.0)
                bT16.append(t16)

            # ---------------- H projections ----------------
            # combined tiles: [0:96] = Ht_nat, [96:128] = Hh_nat rows 32b..
            # Both parts land in ONE psum tile per (block, chunk): the hh
            # rows are matmul'd straight into psum partitions 96..127 via
            # the auto-derived tile_position (M=32 at base 96).
            comb = []
            for b in range(NB):
                t = pp.tile([128, PECOLS], bf16, tag=f"comb{b}")
                comb.append(t)
            def comb_unit(b, cidx):
                lo = cidx * 512
                n = min(512, PECOLS - lo)
                ps = pss.tile([128, 512], f32, tag="tps")
                for d in range(KT):
                    nc.tensor.matmul(
                        ps[0:S, :n], bT16[d][:],
                        wb[:, (KT + d) * H3 + KD * 128 + lo:(KT + d) * H3 + KD * 128 + lo + n],
                        start=(d == 0), stop=False,
                        skip_group_check=True)
                for d in range(KT):
                    nc.tensor.matmul(
                        ps[S:128, :n], bT16[d][:, 32 * b:32 * b + 32],
                        wb[:, d * H3 + KD * 128 + lo:d * H3 + KD * 128 + lo + n],
                        start=(d == 0), stop=False,
                        skip_group_check=True, tile_position=(0, 96))
                nc.tensor.matmul(ps[S:128, :n],
                                 ones16[:, 32 * b:32 * b + 32],
                                 pbnat[:, KD * 128 + lo:KD * 128 + lo + n],
                                 start=False, stop=True,
                                 skip_group_check=True,
                                 tile_position=(0, 96))
                nc.scalar.activation(comb[b][:, lo:lo + n], ps[:, :n],
                                     AF.Identity, scale=1.0)

            for cidx in range(3):
                comb_unit(0, cidx)

            # feature-major hh (f32 + pb) / ht (bf16) for k-tiles 0..5
            hh_fm, ht_fm = [], []
            for k in range(KD):
                msl = slice(k * 128, (k + 1) * 128)
                ps = pss.tile([128, S], f32, tag="tps")
                for d in range(KT):
                    nc.tensor.matmul(ps[:], wb[:, d * H3 + msl.start:d * H3 + msl.stop], bT16[d][:],
                                     start=(d == 0), stop=(d == KT - 1))
                th = pp.tile([128, S], f32, tag=f"hhfm{k}")
                nc.scalar.activation(th[:], ps[:], AF.Identity,
                                     bias=pbfm[:, k:k + 1], scale=1.0)
                hh_fm.append(th)
                ps2 = pss.tile([128, S], f32, tag="tps")
                for d in range(KT):
                    nc.tensor.matmul(ps2[:], wb[:, (KT + d) * H3 + msl.start:(KT + d) * H3 + msl.stop], bT16[d][:],
                                     start=(d == 0), stop=(d == KT - 1))
                tt = pp.tile([128, S], bf16, tag=f"htfm{k}")
                nc.vector.tensor_scalar_mul(tt[:], ps2[:], 1.0)
                ht_fm.append(tt)

            # ---------------- main loop (software-pipelined) ---------------
            def build(g):
                b, gb = g // GPB, g % GPB
                vt = {}
                for k in range(KD, MT):
                    if (k - KD) % 5 < 3:
                        pq = psq.tile([128, NFREE], f32, tag="pairps")
                    else:
                        pq = pss.tile([128, NFREE], f32, tag="tps")
                    nc.tensor.matmul(
                        pq[:], comb[b][:, (k - KD) * 128:(k - KD + 1) * 128],
                        selr[:, gb * NFREE:(gb + 1) * NFREE],
                        start=True, stop=True)
                    v = vp.tile([128, NFREE], bf16, tag=f"v{k}")
                    if k in ACT_COPY_K:
                        nc.scalar.activation(v[:], pq[:], AF.Relu, scale=1.0)
                    else:
                        nc.vector.tensor_scalar_max(v[:], pq[:], 0.0)
                    vt[k] = v
                for k in range(KD):
                    v = vp.tile([128, NFREE], bf16, tag=f"v{k}")
                    for ii in range(IGRP):
                        i = g * IGRP + ii
                        nc.vector.tensor_scalar(
                            v[:, ii * S:(ii + 1) * S], ht_fm[k][:],
                            hh_fm[k][:, i:i + 1], 0.0,
                            op0=ALU.add, op1=ALU.max)
                    vt[k] = v
                return vt

            def mains(g, vt):
                ops = pso.tile([C, NFREE], f32, tag="ops")
                for j, k in enumerate(range(MT)):
                    nc.tensor.matmul(ops[:], rwr[k], vt[k][:],
                                     start=(j == 0), stop=(j == MT - 1))
                ostg = wp.tile([C, NFREE], f32, tag="ostg")
                if g % 2 == 0:
                    nc.scalar.activation(ostg[:], ops[:], AF.Identity,
                                         scale=1.0)
                else:
                    nc.vector.tensor_scalar_mul(ostg[:], ops[:], 1.0)
                nc.sync.dma_start(out[:, g * NFREE:(g + 1) * NFREE], ostg[:])

            prev = build(0)
            if debug:
                stg = wp.tile([128, 3 * NFREE], f32, tag="dbgv")
                nc.vector.tensor_scalar_mul(stg[:, :NFREE], prev[0][:], 1.0)
                nc.vector.tensor_scalar_mul(
                    stg[:, NFREE:2 * NFREE], prev[6][:], 1.0)
                nc.vector.tensor_scalar_mul(
                    stg[:, 2 * NFREE:], prev[17][:], 1.0)
                nc.sync.dma_start(dbg_v[:], stg[:])
                stg2 = wp.tile([128, PECOLS], f32, tag="dbgc")
                nc.vector.tensor_scalar_mul(stg2[:], comb[0][:], 1.0)
                nc.sync.dma_start(dbg_comb[:], stg2[:])
                stg3 = wp.tile([RSEQ, S + 8], f32, tag="dbgs")
                nc.vector.tensor_scalar_mul(stg3[:, :S], wsum[:], 1.0)
                nc.vector.tensor_scalar_mul(stg3[:, S:], g_sb[:], 1.0)
                nc.sync.dma_start(dbg_scan[:], stg3[:])
                stg4 = wp.tile([128, 2 * S], f32, tag="dbgf")
                nc.vector.tensor_scalar_mul(stg4[:, :S], hh_fm[0][:], 1.0)
                nc.vector.tensor_scalar_mul(stg4[:, S:], ht_fm[0][:], 1.0)
                nc.sync.dma_start(dbg_fm[:], stg4[:])
            for g in range(1, NG):
                if g in (2, 4, 6) or g in (10, 12, 14):
                    comb_unit(1 if g < 8 else 2, (g % 8) // 2 - 1)
                cur = build(g)
                mains(g - 1, prev)
                prev = cur
            mains(NG - 1, prev)

    nc.finalize()
    return nc


_CACHED_NC = None


def _host_consts():
    import ml_dtypes
    bf = ml_dtypes.bfloat16
    sel = np.zeros((128, GPB * NFREE), np.float32)
    for gb in range(GPB):
        base = gb * NFREE
        for ii in range(IGRP):
            sel[np.arange(S), base + ii * S + np.arange(S)] = 1.0
            sel[S + gb * IGRP + ii, base + ii * S:base + (ii + 1) * S] = 1.0
    return sel.astype(bf)


def _prep_in_maps(encoded_text, rel_types_encoded, proj_W, proj_b, rel_W):
    import ml_dtypes
    bf = ml_dtypes.bfloat16
    relw_perm = np.ascontiguousarray(
        rel_W.reshape(H3, R, TAG).transpose(0, 2, 1).reshape(H3, C)
    ).astype(bf)
    pw16 = np.ascontiguousarray(proj_W).astype(bf)
    selr = _host_consts()
    pb32 = np.asarray(proj_b, dtype=np.float32)
    pbfm = np.ascontiguousarray(pb32.reshape(MT, 128).T)  # [128, MT]
    pbnat = pb32.reshape(1, H3).astype(bf)
    in_maps = []
    for i in range(B):
        in_maps.append({
            "enc": np.ascontiguousarray(encoded_text[i], dtype=np.float32),
            "arel": np.ascontiguousarray(
                rel_types_encoded[i], dtype=np.float32),
            "pw16": pw16,
            "relw16": relw_perm,
            "selr": selr,
            "pbfm": pbfm,
            "pbnat": pbnat,
        })
    return in_maps


def _assemble(results, rel_b):
    outs = []
    for i in range(B):
        o = results[i]["out"].reshape(TAG, R, S, S)
        outs.append(o)
    full = np.stack(outs, axis=0).astype(np.float32)
    if np.any(rel_b):
        relb_perm = np.asarray(rel_b, dtype=np.float32).reshape(R, TAG).T
        full = full + relb_perm[None, :, :, None, None]
    return full


def kernel(encoded_text, rel_types_encoded, proj_W, proj_b, rel_W, rel_b):
    global _CACHED_NC
    from concourse.bass_utils import run_bass_kernel_spmd

    if _CACHED_NC is None:
        _CACHED_NC = build_nc()
    in_maps = _prep_in_maps(
        encoded_text, rel_types_encoded, proj_W, proj_b, rel_W)
    res = run_bass_kernel_spmd(_CACHED_NC, in_maps, list(range(B)))
    return _assemble(res.results, rel_b)


# revision 17
# speedup vs baseline: 1.4014x; 1.0202x over previous
"""Trainium2 Bass kernel for nn_AttModel (B=8, S=96, D=768, R=24, RSEQ=8, TAG=3).

Data-parallel over batch: core i handles sample i. v2 design:

  1. Host pre-converts proj_W / rel_W(permuted) / pair-selectors to bf16;
     W DMA is ~21us instead of 42us.
  2. Refine scan in score space with LAGGED max (reduce_max runs off the
     critical chain; exp(s_t - max(s_{t-1})) <= e^30, safe in f32) and the
     softmax normalizer folded into the tiny [8,8] G-scale instead of a
     [8,96] w-scale.
  3. H projections in bf16, two layouts:
     - feature-major hh/ht [128, 96] for k-tiles 0..5 (DVE-direct V build)
     - natural-layout combined tiles [128, 1536] for k-tiles 6..17:
       partitions 0..95 = Ht_nat(j), 96..127 = Hh_nat rows 32b..32b+31
  4. Main loop per group g (4 i's x 96 j's = 384 pairs):
     - k 6..17: pair-matmul  P = combined[:, kslice].T @ selR[g%8]  on PE,
       then relu PSUM->SBUF bf16 copy (k 6..15 on ACT, 16..17 on DVE)
     - k 0..5: 4x per-i tensor_scalar add+relu on DVE
     - 18 accumulating main matmuls rwr[k].T @ V[k] -> out psum [72, 384]
     Software-pipelined: build(g+1) is emitted before mains(g) so the PE
     never stalls on the relu copies.
Output per core: [72, 9216] with channel c = tag*24 + rel (rel_W pre-permuted
on host), reshaped on host to [3, 24, 96, 96].
"""
import sys

sys.path.insert(0, "/opt/trn_rl_repo")

import numpy as np

S, D, H3 = 96, 768, 2304
R, RSEQ, TAG, C = 24, 8, 3, 72
B = 8
KT = D // 128           # 6 d-chunks per half of proj_W
MT = H3 // 128          # 18 feature tiles
KD = 7                  # k-tiles 0..KD-1 built DVE-direct
KP = MT - KD            # k-tiles 6..17 built via pair-matmul
PECOLS = KP * 128       # 1536 features in combined tiles
IGRP = 4
NG = S // IGRP          # 24 groups
NFREE = IGRP * S        # 384
NB = S // 32            # 3 combined blocks (32 i's each)
GPB = 32 // IGRP        # 8 groups per block
ACT_COPY_K = set(range(KD, KD + 10))   # pair-copy on ACT; rest on DVE
SCALE = 1.0 / float(np.sqrt(np.float32(D)))


def build_nc(repeat: int = 1, debug: bool = False):
    import concourse.bass as bass
    from concourse import bacc, mybir
    import concourse.tile as tile
    from concourse.masks import make_identity

    f32 = mybir.dt.float32
    bf16 = mybir.dt.bfloat16
    AF = mybir.ActivationFunctionType
    ALU = mybir.AluOpType
    AX = mybir.AxisListType

    nc = bacc.Bacc()
    enc = nc.dram_tensor("enc", [S, D], f32, kind="ExternalInput")
    arel = nc.dram_tensor("arel", [RSEQ, D], f32, kind="ExternalInput")
    pw16 = nc.dram_tensor("pw16", [2 * D, H3], bf16, kind="ExternalInput")
    relw16 = nc.dram_tensor("relw16", [H3, C], bf16, kind="ExternalInput")
    selr_d = nc.dram_tensor("selr", [128, GPB * NFREE], bf16,
                            kind="ExternalInput")
    pbfm_d = nc.dram_tensor("pbfm", [128, MT], f32, kind="ExternalInput")
    pbnat_d = nc.dram_tensor("pbnat", [1, H3], bf16, kind="ExternalInput")
    out = nc.dram_tensor("out", [C, S * S], f32, kind="ExternalOutput")
    if debug:
        dbg_comb = nc.dram_tensor("dbg_comb", [128, PECOLS], f32,
                                  kind="ExternalOutput")
        dbg_v = nc.dram_tensor("dbg_v", [128, 3 * NFREE], f32,
                               kind="ExternalOutput")
        dbg_scan = nc.dram_tensor("dbg_scan", [RSEQ, S + 8], f32,
                                  kind="ExternalOutput")
        dbg_fm = nc.dram_tensor("dbg_fm", [128, 2 * S], f32,
                                kind="ExternalOutput")

    with tile.TileContext(nc) as tc:
        with (
            tc.tile_pool(name="persist", bufs=1) as pp,
            tc.tile_pool(name="work", bufs=2) as wp,
            tc.tile_pool(name="vpool", bufs=3) as vp,
            tc.tile_pool(name="scanp", bufs=2) as sp,
            tc.tile_pool(name="psmall", bufs=2, space="PSUM") as pss,
            tc.tile_pool(name="psone", bufs=1, space="PSUM") as ps1,
            tc.tile_pool(name="pspair", bufs=3, space="PSUM") as psq,
            tc.tile_pool(name="psout", bufs=2, space="PSUM") as pso,
        ):
            # ---------------- loads ----------------
            ident = pp.tile([128, 128], f32, tag="ident")
            make_identity(nc, ident[:])

            enc_nat = pp.tile([S, D], f32, tag="enc_nat")
            nc.sync.dma_start(enc_nat[:], enc[:])
            a_nat = pp.tile([RSEQ, D], f32, tag="a_nat")
            nc.sync.dma_start(a_nat[:], arel[:])
            selr = pp.tile([128, GPB * NFREE], bf16, tag="selr")
            nc.sync.dma_start(selr[:], selr_d[:])
            pbfm = pp.tile([128, MT], f32, tag="pbfm")
            nc.sync.dma_start(pbfm[:], pbfm_d[:])
            pbnat = pp.tile([1, H3], bf16, tag="pbnat")
            nc.sync.dma_start(pbnat[:], pbnat_d[:])
            rwrb = pp.tile([128, MT * C], bf16, tag="rwrb")
            nc.sync.dma_start(
                rwrb[:].rearrange("p (k c) -> p k c", k=MT),
                relw16.rearrange("(k p) c -> p k c", p=128))
            rwr = [rwrb[:, k * C:(k + 1) * C] for k in range(MT)]
            wb = pp.tile([128, 2 * KT * H3], bf16, tag="wb")
            nc.sync.dma_start(
                wb[:].rearrange("p (n m) -> p n m", n=2 * KT),
                pw16.rearrange("(n p) m -> p n m", p=128))
            w16 = [wb[:, d * H3:(d + 1) * H3] for d in range(2 * KT)]

            ones16 = pp.tile([1, S], bf16, tag="ones16")
            nc.vector.memset(ones16[:], 1.0)

            # ---------------- transposes / scan prep (f32) ----------------
            bT = []
            for k in range(KT):
                if k % 2 == 0:
                    ps = psq.tile([128, S], f32, tag="pairps")
                else:
                    ps = pss.tile([128, S], f32, tag="tps")
                nc.tensor.transpose(
                    ps[:], enc_nat[:, k * 128:(k + 1) * 128], ident[:S, :S])
                t = pp.tile([128, S], f32, tag=f"bT{k}")
                nc.scalar.copy(t[:], ps[:])
                bT.append(t)
            at_scl = []
            for k in range(KT):
                if k % 2 == 0:
                    ps = psq.tile([128, RSEQ], f32, tag="pairps")
                else:
                    ps = pss.tile([128, RSEQ], f32, tag="tps")
                nc.tensor.transpose(
                    ps[:], a_nat[:, k * 128:(k + 1) * 128],
                    ident[:RSEQ, :RSEQ])
                ts = pp.tile([128, RSEQ], f32, tag=f"ats{k}")
                nc.scalar.mul(ts[:], ps[:], SCALE)
                at_scl.append(ts)

            gps = pss.tile([RSEQ, RSEQ], f32, tag="tps")
            for k in range(KT):
                nc.tensor.matmul(gps[:], at_scl[k][:], at_scl[k][:],
                                 start=(k == 0), stop=(k == KT - 1))
            # gps = scale^2 * A@A.T; fold one 1/scale back in
            g_sb = pp.tile([RSEQ, RSEQ], f32, tag="g_sb")
            nc.vector.tensor_scalar_mul(g_sb[:], gps[:], 1.0 / SCALE)

            s_ps = ps1.tile([RSEQ, S], f32, tag="s_ps")
            for k in range(KT):
                nc.tensor.matmul(s_ps[:], at_scl[k][:], bT[k][:],
                                 start=(k == 0), stop=False,
                                 skip_group_check=True)

            # ---------------- scan (lagged max, G-folded normalizer) -------
            wsum = pp.tile([RSEQ, S], f32, tag="wsum")
            nc.vector.memset(wsum[:], 0.0)
            negmax = sp.tile([RSEQ, 1], f32, tag="negmax")
            nc.vector.reduce_max(negmax[:], s_ps[:], axis=AX.X, negate=True)
            for t in range(R):
                u = sp.tile([RSEQ, S], f32, tag="u")
                rs = sp.tile([RSEQ, 1], f32, tag="rs")
                nc.scalar.activation(u[:], s_ps[:], AF.Exp, bias=negmax[:],
                                     scale=1.0, accum_out=rs[:])
                rinv = sp.tile([RSEQ, 1], f32, tag="rinv")
                nc.vector.reciprocal(rinv[:], rs[:])
                if t < R - 1:
                    # critical chain: gsc = G * rinv, then the matmul.
                    gsc = sp.tile([RSEQ, RSEQ], f32, tag="gsc")
                    nc.vector.tensor_scalar_mul(gsc[:], g_sb[:], rinv[:])
                    # lagged max: read s_t BEFORE the matmul updates it; the
                    # next exp sees s_{t+1} - max(s_t) <= ~30, safe in f32.
                    negmax = sp.tile([RSEQ, 1], f32, tag="negmax")
                    nc.vector.reduce_max(negmax[:], s_ps[:], axis=AX.X,
                                         negate=True)
                    nc.tensor.matmul(s_ps[:], gsc[:], u[:],
                                     start=False, stop=(t == R - 2),
                                     skip_group_check=True)
                # wsum += u * rinv (off critical chain)
                nc.vector.scalar_tensor_tensor(
                    wsum[:], u[:], rinv[:], wsum[:],
                    op0=ALU.mult, op1=ALU.add)

            # ---------------- b update + bf16 ----------------
            bT16 = []
            for k in range(KT):
                ps = pss.tile([128, S], f32, tag="tps")
                nc.tensor.matmul(ps[:], a_nat[:, k * 128:(k + 1) * 128],
                                 wsum[:], start=True, stop=True)
                nc.vector.tensor_tensor(bT[k][:], bT[k][:], ps[:], op=ALU.add)
                t16 = pp.tile([128, S], bf16, tag=f"bT16_{k}")
                nc.vector.tensor_scalar_mul(t16[:], bT[k][:], 1.0)
                bT16.append(t16)

            # ---------------- H projections ----------------
            # combined tiles: [0:96] = Ht_nat, [96:128] = Hh_nat rows 32b..
            # Both parts land in ONE psum tile per (block, chunk): the hh
            # rows are matmul'd straight into psum partitions 96..127 via
            # the auto-derived tile_position (M=32 at base 96).
            comb = []
            for b in range(NB):
                t = pp.tile([128, PECOLS], bf16, tag=f"comb{b}")
                comb.append(t)
            def comb_unit(b, cidx):
                lo = cidx * 512
                n = min(512, PECOLS - lo)
                ps = pss.tile([128, 512], f32, tag="tps")
                for d in range(KT):
                    nc.tensor.matmul(
                        ps[0:S, :n], bT16[d][:],
                        wb[:, (KT + d) * H3 + KD * 128 + lo:(KT + d) * H3 + KD * 128 + lo + n],
                        start=(d == 0), stop=False,
                        skip_group_check=True)
                for d in range(KT):
                    nc.tensor.matmul(
                        ps[S:128, :n], bT16[d][:, 32 * b:32 * b + 32],
                        wb[:, d * H3 + KD * 128 + lo:d * H3 + KD * 128 + lo + n],
                        start=(d == 0), stop=False,
                        skip_group_check=True, tile_position=(0, 96))
                nc.tensor.matmul(ps[S:128, :n],
                                 ones16[:, 32 * b:32 * b + 32],
                                 pbnat[:, KD * 128 + lo:KD * 128 + lo + n],
                                 start=False, stop=True,
                                 skip_group_check=True,
                                 tile_position=(0, 96))
                nc.scalar.activation(comb[b][:, lo:lo + n], ps[:, :n],
                                     AF.Identity, scale=1.0)

            for cidx in range(3):
                comb_unit(0, cidx)

            # feature-major hh (f32 + pb) / ht (bf16) for k-tiles 0..5
            hh_fm, ht_fm = [], []
            for k in range(KD):
                msl = slice(k * 128, (k + 1) * 128)
                ps = pss.tile([128, S], f32, tag="tps")
                for d in range(KT):
                    nc.tensor.matmul(ps[:], wb[:, d * H3 + msl.start:d * H3 + msl.stop], bT16[d][:],
                                     start=(d == 0), stop=(d == KT - 1))
                th = pp.tile([128, S], f32, tag=f"hhfm{k}")
                nc.scalar.activation(th[:], ps[:], AF.Identity,
                                     bias=pbfm[:, k:k + 1], scale=1.0)
                hh_fm.append(th)
                ps2 = pss.tile([128, S], f32, tag="tps")
                for d in range(KT):
                    nc.tensor.matmul(ps2[:], wb[:, (KT + d) * H3 + msl.start:(KT + d) * H3 + msl.stop], bT16[d][:],
                                     start=(d == 0), stop=(d == KT - 1))
                tt = pp.tile([128, S], bf16, tag=f"htfm{k}")
                nc.vector.tensor_scalar_mul(tt[:], ps2[:], 1.0)
                ht_fm.append(tt)

            # ---------------- main loop (software-pipelined) ---------------
            def build(g):
                b, gb = g // GPB, g % GPB
                vt = {}
                for k in range(KD, MT):
                    if (k - KD) % 5 < 3:
                        pq = psq.tile([128, NFREE], f32, tag="pairps")
                    else:
                        pq = pss.tile([128, NFREE], f32, tag="tps")
                    nc.tensor.matmul(
                        pq[:], comb[b][:, (k - KD) * 128:(k - KD + 1) * 128],
                        selr[:, gb * NFREE:(gb + 1) * NFREE],
                        start=True, stop=True)
                    v = vp.tile([128, NFREE], bf16, tag=f"v{k}")
                    if k in ACT_COPY_K:
                        nc.scalar.activation(v[:], pq[:], AF.Relu, scale=1.0)
                    else:
                        nc.vector.tensor_scalar_max(v[:], pq[:], 0.0)
                    vt[k] = v
                for k in range(KD):
                    v = vp.tile([128, NFREE], bf16, tag=f"v{k}")
                    for ii in range(IGRP):
                        i = g * IGRP + ii
                        nc.vector.tensor_scalar(
                            v[:, ii * S:(ii + 1) * S], ht_fm[k][:],
                            hh_fm[k][:, i:i + 1], 0.0,
                            op0=ALU.add, op1=ALU.max)
                    vt[k] = v
                return vt

            def mains(g, vt):
                ops = pso.tile([C, NFREE], f32, tag="ops")
                for j, k in enumerate(range(MT)):
                    nc.tensor.matmul(ops[:], rwr[k], vt[k][:],
                                     start=(j == 0), stop=(j == MT - 1))
                ostg = wp.tile([C, NFREE], f32, tag="ostg")
                if g % 2 == 0:
                    nc.scalar.activation(ostg[:], ops[:], AF.Identity,
                                         scale=1.0)
                else:
                    nc.vector.tensor_scalar_mul(ostg[:], ops[:], 1.0)
                nc.sync.dma_start(out[:, g * NFREE:(g + 1) * NFREE], ostg[:])

            prev = build(0)
            if debug:
                stg = wp.tile([128, 3 * NFREE], f32, tag="dbgv")
                nc.vector.tensor_scalar_mul(stg[:, :NFREE], prev[0][:], 1.0)
                nc.vector.tensor_scalar_mul(
                    stg[:, NFREE:2 * NFREE], prev[6][:], 1.0)
                nc.vector.tensor_scalar_mul(
                    stg[:, 2 * NFREE:], prev[17][:], 1.0)
                nc.sync.dma_start(dbg_v[:], stg[:])
                stg2 = wp.tile([128, PECOLS], f32, tag="dbgc")
                nc.vector.tensor_scalar_mul(stg2[:], comb[0][:], 1.0)
                nc.sync.dma_start(dbg_comb[:], stg2[:])
                stg3 = wp.tile([RSEQ, S + 8], f32, tag="dbgs")
                nc.vector.tensor_scalar_mul(stg3[:, :S], wsum[:], 1.0)
                nc.vector.tensor_scalar_mul(stg3[:, S:], g_sb[:], 1.0)
                nc.sync.dma_start(dbg_scan[:], stg3[:])
                stg4 = wp.tile([128, 2 * S], f32, tag="dbgf")
                nc.vector.tensor_scalar_mul(stg4[:, :S], hh_fm[0][:], 1.0)
                nc.vector.tensor_scalar_mul(stg4[:, S:], ht_fm[0][:], 1.0)
                nc.sync.dma_start(dbg_fm[:], stg4[:])
            for g in range(1, NG):
                if g in (2, 4, 6) or g in (10, 12, 14):
                    comb_unit(1 if g < 8 else 2, (g % 8) // 2 - 1)
                cur = build(g)
                mains(g - 1, prev)
                prev = cur
            mains(NG - 1, prev)

    nc.finalize()
    return nc


_CACHED_NC = None


def _host_consts():
    import ml_dtypes
    bf = ml_dtypes.bfloat16
    sel = np.zeros((128, GPB * NFREE), np.float32)
    for gb in range(GPB):
        base = gb * NFREE
        for ii in range(IGRP):
            sel[np.arange(S), base + ii * S + np.arange(S)] = 1.0
            sel[S + gb * IGRP + ii, base + ii * S:base + (ii + 1) * S] = 1.0
    return sel.astype(bf)


def _prep_in_maps(encoded_text, rel_types_encoded, proj_W, proj_b, rel_W):
    import ml_dtypes
    bf = ml_dtypes.bfloat16
    relw_perm = np.ascontiguousarray(
        rel_W.reshape(H3, R, TAG).transpose(0, 2, 1).reshape(H3, C)
    ).astype(bf)
    pw16 = np.ascontiguousarray(proj_W).astype(bf)
    selr = _host_consts()
    pb32 = np.asarray(proj_b, dtype=np.float32)
    pbfm = np.ascontiguousarray(pb32.reshape(MT, 128).T)  # [128, MT]
    pbnat = pb32.reshape(1, H3).astype(bf)
    in_maps = []
    for i in range(B):
        in_maps.append({
            "enc": np.ascontiguousarray(encoded_text[i], dtype=np.float32),
            "arel": np.ascontiguousarray(
                rel_types_encoded[i], dtype=np.float32),
            "pw16": pw16,
            "relw16": relw_perm,
            "selr": selr,
            "pbfm": pbfm,
            "pbnat": pbnat,
        })
    return in_maps


def _assemble(results, rel_b):
    outs = []
    for i in range(B):
        o = results[i]["out"].reshape(TAG, R, S, S)
        outs.append(o)
    full = np.stack(outs, axis=0).astype(np.float32)
    if np.any(rel_b):
        relb_perm = np.asarray(rel_b, dtype=np.float32).reshape(R, TAG).T
        full = full + relb_perm[None, :, :, None, None]
    return full


def kernel(encoded_text, rel_types_encoded, proj_W, proj_b, rel_W, rel_b):
    global _CACHED_NC
    from concourse.bass_utils import run_bass_kernel_spmd

    if _CACHED_NC is None:
        _CACHED_NC = build_nc()
    in_maps = _prep_in_maps(
        encoded_text, rel_types_encoded, proj_W, proj_b, rel_W)
    res = run_bass_kernel_spmd(_CACHED_NC, in_maps, list(range(B)))
    return _assemble(res.results, rel_b)
